# revision 18
# baseline (speedup 1.0000x reference)
"""Conv4d (Strang rearrange) Trainium2 kernel — raw bacc pipeline, v3.

Per core (b, D1-half): 32 groups g=(u, rnd), each 9 shifts x 4 col-tiled
matmuls (ku-major order) accumulating into one of 8 psum banks; vector
engine does psum+bias -> fp16 out slots; sync+scalar act as two DMA queues.

v3 vs v2:
  - PE pre-warm: dummy matmuls on scratch SBUF during the initial DMA wait
    release the HAM clock gate (1.2 -> 2.4 GHz) before real work arrives.
  - bias rides the tail of the weight DMA (bf16 column 288 of wt_ext).
  - Startup-critical DMAs (w + rows 0-2, both halves) split across both
    queues by partition halves; one sem per row-half, final-value waits.
  - Row halves a on sync, b on scalar; outputs interleaved by deadline.
  - No trailing drain waits; runtime epilogue quiesces DMA.
"""

from contextlib import ExitStack

import ml_dtypes
import numpy as np

import concourse.bass as bass
from concourse import bacc, mybir
from concourse.bass_utils import run_bass_kernel_spmd

F16 = mybir.dt.float16
BF16 = mybir.dt.bfloat16
F32 = mybir.dt.float32

B, CIN, COUT = 4, 4, 4
D1, D2, H, W = 32, 32, 64, 64
U = 16
R = U + 2
V = D2
I, J = H // 2, W // 2
IB, IO = 8, 4
VBS = 4
NCORES = 8
NZ, NPS, NOUT = 10, 8, 8
NG = 2 * U  # 32 groups
NWARM = 12
WCOLS = 288  # 9*32 weight cols

SHIFTS = [(ku, kv) for ku in range(3) for kv in (1, 0, 2)]
NSHIFT = len(SHIFTS)

# Queue orders: single source of truth for DMA issue order per engine.
# w0/w1 = weight+bias halves (partitions 0:64 / 64:128); rNa0/rNa1 = row N
# v[0:17] partition halves; rNa/rNb = full-partition row halves; oG = output.
Q_SYNC = ["w0", "r0a0", "r1a0", "r2a0", "b", "r0b0", "r1b0", "r2b0",
          "r3a", "r4a", "o0", "r5a", "o2", "r6a", "o4", "r7a", "o6",
          "r8a", "o8", "r9a", "o10", "r10a", "o12", "r11a", "o14",
          "r12a", "o16", "r13a", "o18", "r14a", "o20", "r15a", "o22",
          "r16a", "o24", "r17a", "o26", "o28", "o30"]
Q_SCAL = ["w1", "r0a1", "r1a1", "r2a1", "r0b1", "r1b1", "r2b1",
          "r3b", "r4b", "o1", "r5b", "o3", "r6b", "o5", "r7b", "o7",
          "r8b", "o9", "r9b", "o11", "r10b", "o13", "r11b", "o15",
          "r12b", "o17", "r13b", "o19", "r14b", "o21", "r15b", "o23",
          "r16b", "o25", "r17b", "o27", "o29", "o31"]


def _host_weights(w, b):
    wbd = np.zeros((NSHIFT, 128, 32), np.float32)
    w = np.asarray(w, np.float32)
    for s, (ku, kv) in enumerate(SHIFTS):
        for kh in range(2):
            for kw in range(2):
                for ib in range(IB):
                    wbd[s, kh * 16 + kw * 8 + ib : 128 : 32, ib : 32 : 8] = (
                        w[:, :, ku, kv, kh, kw].T
                    )
    wext = wbd.transpose(1, 0, 2).reshape(128, NSHIFT * 32)
    bias = np.tile(np.repeat(np.asarray(b, np.float32), IB), 4).reshape(128, 1)
    return np.ascontiguousarray(wext).astype(ml_dtypes.bfloat16), bias


def _host_shard(x):
    xp = np.pad(np.asarray(x, np.float32), ((0, 0), (0, 0), (1, 1), (0, 0), (0, 0), (0, 0)))
    shards = []
    for core in range(NCORES):
        bb, half = divmod(core, 2)
        xs = xp[bb, :, half * U : half * U + R]
        xs = xs.reshape(CIN, R, V, IO, IB, 2, J, 2)
        xs = xs.transpose(1, 0, 5, 7, 4, 2, 3, 6).astype(ml_dtypes.bfloat16)
        shards.append(np.ascontiguousarray(xs).reshape(R, 128, V, IO, J))
    return shards


def _build_program():
    nc = bacc.Bacc("TRN2", target_bir_lowering=False, debug=False)
    xs = nc.dram_tensor("xs", [R, 128, V, IO, J], BF16, kind="ExternalInput").ap()
    wbd = nc.dram_tensor("wbd", [128, WCOLS], BF16, kind="ExternalInput").ap()
    bias = nc.dram_tensor("bias", [128, 1], F32, kind="ExternalInput").ap()
    ys = nc.dram_tensor("ys", [U, 2, 128, VBS, IO, J], F16, kind="ExternalOutput").ap()

    with ExitStack() as ctx:
        zt = ctx.enter_context(nc.sbuf_tensor("zt", [128, NZ, V, IO, J], BF16))
        wt = ctx.enter_context(nc.sbuf_tensor("wt", [128, WCOLS], BF16))
        bt = ctx.enter_context(nc.sbuf_tensor("bt", [128, 1], F32))
        ot = ctx.enter_context(nc.sbuf_tensor("ot", [128, NOUT, VBS, IO, J], F16))
        junk = ctx.enter_context(nc.sbuf_tensor("junk", [128, 544], BF16))
        ps = [ctx.enter_context(nc.psum_tensor(f"ps{i}", [128, VBS, IO, J], F32)) for i in range(NPS)]
        # One semaphore per DMA target; split dmas inc the same sem and are
        # only ever waited at their final value (16 per contributing dma).
        sem_za = [ctx.enter_context(nc.semaphore(f"sem_za{r}")) for r in range(R)]
        sem_zb = [ctx.enter_context(nc.semaphore(f"sem_zb{r}")) for r in range(R)]
        sem_w = ctx.enter_context(nc.semaphore("sem_w"))
        sem_b = ctx.enter_context(nc.semaphore("sem_b"))
        sem_mm = ctx.enter_context(nc.semaphore("sem_mm"))
        sem_act = ctx.enter_context(nc.semaphore("sem_act"))
        sem_os = [ctx.enter_context(nc.semaphore(f"sem_o{g}")) for g in range(NG)]
        sem_warm = ctx.enter_context(nc.semaphore("sem_warm"))
        sem_junk = ctx.enter_context(nc.semaphore("sem_junk"))

        def emit_queue(eng, order):
            for item in order:
                if item == "b":
                    eng.dma_start(bt[:], bias[:]).then_inc(sem_b, 16)
                elif item.startswith("w"):
                    p0 = 0 if item == "w0" else 64
                    eng.dma_start(wt[p0 : p0 + 64], wbd[p0 : p0 + 64]).then_inc(sem_w, 16)
                elif item.startswith("o"):
                    g = int(item[1:])
                    u, rnd = divmod(g, 2)
                    eng.wait_ge(sem_act, g + 1)
                    eng.dma_start(ys[u, rnd], ot[:, g % NOUT]).then_inc(sem_os[g], 16)
                else:
                    body = item[1:]
                    half = None
                    if body.endswith(("0", "1")):
                        half, body = int(body[-1]), body[:-1]
                    r = int(body[:-1])
                    ab = body[-1]
                    if r >= NZ:
                        eng.wait_ge(sem_mm, 2 * (r - NZ) + 2)
                    vs = slice(0, 17) if ab == "a" else slice(17, V)
                    psl = slice(None) if half is None else slice(half * 64, half * 64 + 64)
                    sem = sem_za[r] if ab == "a" else sem_zb[r]
                    eng.dma_start(zt[psl, r % NZ, vs], xs[r, psl, vs]).then_inc(sem, 16)

        blk_ctx = nc.Block()
        block = blk_ctx.__enter__()

        @block.sync
        def _(sync):
            emit_queue(sync, Q_SYNC)

        @block.scalar
        def _(scalar):
            emit_queue(scalar, Q_SCAL)

        @block.gpsimd
        def _(gpsimd):
            nc.gpsimd.memset(junk[:], 0).then_inc(sem_junk)

        @block.tensor
        def _(tensor):
            # HAM pre-warm: keep the PE busy on scratch data while the first
            # z rows stream in, so real groups run at the warm 2.4 GHz clock.
            tensor.wait_ge(sem_junk, 1)
            last = None
            for k in range(NWARM):
                last = nc.tensor.matmul(
                    ps[NPS - 1][0:32],
                    junk[:, 0:32],
                    junk[:, 32 : 32 + 512],
                    start=(k == 0),
                    stop=(k == NWARM - 1),
                    skip_group_check=True,
                    tile_position=(0, 0),
                )
            last.then_inc(sem_warm)
            for g in range(NG):
                u, rnd = divmod(g, 2)
                if g == 0:
                    tensor.wait_ge(sem_w, 32)
                    tensor.wait_ge(sem_za[0], 32)
                    tensor.wait_ge(sem_za[1], 32)
                if g == 1:
                    tensor.wait_ge(sem_zb[0], 32)
                    tensor.wait_ge(sem_zb[1], 32)
                if g == NPS - 1:
                    tensor.wait_ge(sem_warm, 1)
                if g >= NPS:
                    tensor.wait_ge(sem_act, g - NPS + 1)
                # new-row wait deferred to just before the first ku=2 matmul
                r_new = u + 2
                sem_new = sem_za[r_new] if rnd == 0 else sem_zb[r_new]
                late = (sem_new, 32 if r_new <= 2 else 16)
                psg = ps[g % NPS]
                last = None
                for s, (ku, kv) in enumerate(SHIFTS):
                    if ku == 2 and s % 3 == 0:
                        tensor.wait_ge(*late)
                    for c in range(4):
                        v0 = (rnd * 4 + c) * VBS
                        vv0 = max(0, 1 - kv - v0)
                        vv1 = min(VBS, V + 1 - kv - v0)
                        a = v0 + vv0 + kv - 1
                        last = nc.tensor.matmul(
                            psg[c * 32 : (c + 1) * 32, vv0:vv1, :, :],
                            wt[:, s * 32 : (s + 1) * 32],
                            zt[:, (u + ku) % NZ, a : a + (vv1 - vv0), :, :],
                            start=(s == 0),
                            stop=(s == NSHIFT - 1),
                            skip_group_check=True,
                            tile_position=(0, c * 32),
                        )
                last.then_inc(sem_mm)

        @block.vector
        def _(vector):
            bt_ap = bt[:]
            for g in range(NG):
                if g == 0:
                    vector.wait_ge(sem_b, 16)
                if g >= NOUT:
                    vector.wait_ge(sem_os[g - NOUT], 16)
                vector.wait_ge(sem_mm, g + 1)
                nc.vector.tensor_scalar(
                    ot[:, g % NOUT],
                    ps[g % NPS][:],
                    bt_ap,
                    None,
                    mybir.AluOpType.add,
                ).then_inc(sem_act)

        blk_ctx.__exit__(None, None, None)

    nc.compile()
    return nc


def _unshard(results):
    y = np.empty((B, COUT, D1, D2, I, J), np.float32)
    for core in range(NCORES):
        bb, half = divmod(core, 2)
        arr = results[core]["ys"].astype(np.float32).reshape(U, 2, 4, COUT, IB, VBS, IO, J)
        arr = arr.transpose(3, 0, 1, 2, 5, 6, 4, 7)
        y[bb, :, half * U : (half + 1) * U] = arr.reshape(COUT, U, V, I, J)
    return y


TRACE = False
LAST_RESULT = [None]


def kernel(x, w, b, _cache={}):
    if "nc" not in _cache:
        _cache["nc"] = _build_program()
    nc = _cache["nc"]
    wext, bias_h = _host_weights(w, b)
    in_maps = [{"xs": xs, "wbd": wext, "bias": bias_h} for xs in _host_shard(x)]
    res = run_bass_kernel_spmd(nc, in_maps, list(range(NCORES)), trace=TRACE)
    LAST_RESULT[0] = res
    return _unshard(res.results)


# revision 22
# speedup vs baseline: 1.0242x; 1.0242x over previous
"""Conv4d (Strang rearrange) Trainium2 kernel — raw bacc pipeline, v4.

Per core (b, D1-half): 32 groups g=(u, rnd), each 9 shifts x 4 col-tiled
matmuls (ku-major order) accumulating into one of 8 psum banks; vector
engine does psum+bias -> fp16 out slots; sync+scalar act as two DMA queues.

Key scheduling ideas:
  - PE pre-warm: dummy matmuls on scratch SBUF during the initial DMA wait
    release the HAM clock gate (1.2 -> 2.4 GHz) and keep it released until
    real data lands, so every real group runs at the warm clock.
  - ku-major shifts: the row-(u+2) wait sits 6 matmuls into each group.
  - The f32 bias rides as two raw uint16 columns at the tail of the weight
    DMA (AP bitcast), so no separate bias DMA on the critical path.
  - Queue fronts ordered so g0's rows (0,1,2) land just-in-time across both
    queues; one semaphore per DMA, waited only at final values.
  - Outputs interleaved by deadline behind rows; explicit psum-slot WAR
    waits for rows >= NZ.
"""

from contextlib import ExitStack

import ml_dtypes
import numpy as np

import concourse.bass as bass
from concourse import bacc, mybir
from concourse.bass_utils import run_bass_kernel_spmd

F16 = mybir.dt.float16
BF16 = mybir.dt.bfloat16
F32 = mybir.dt.float32

B, CIN, COUT = 4, 4, 4
D1, D2, H, W = 32, 32, 64, 64
U = 16
R = U + 2
V = D2
I, J = H // 2, W // 2
IB, IO = 8, 4
VBS = 4
NCORES = 8
NZ, NPS, NOUT = 10, 8, 8
NG = 2 * U  # 32 groups
NWARM = 44
WCOLS = 292  # 9*32 weight cols + 2 bias-bit cols + 2 pad

SHIFTS = [(ku, kv) for ku in range(3) for kv in (1, 0, 2)]
NSHIFT = len(SHIFTS)

# Queue orders: single source of truth for DMA issue order per engine.
# rNa/rNb = row N v[0:17] / v[17:32]; oG = output of group G; w = weights+bias.
Q_SYNC = ["w", "r0a", "r0b", "r4a", "r4b", "o0", "r6a", "r6b", "o2", "o4",
          "r8a", "r8b", "o6", "o8", "r10a", "r10b", "o10", "o12",
          "r12a", "r12b", "o14", "o16", "r14a", "r14b", "o18", "o20",
          "r16a", "r16b", "o22", "o24", "o26", "o28", "o30"]
Q_SCAL = ["r1a", "r2a", "r1b", "r2b", "r3a", "r3b", "r5a", "r5b", "o1",
          "r7a", "r7b", "o3", "o5", "r9a", "r9b", "o7", "o9",
          "r11a", "r11b", "o11", "o13", "r13a", "r13b", "o15", "o17",
          "r15a", "r15b", "o19", "o21", "r17a", "r17b", "o23", "o25",
          "o27", "o29", "o31"]


def _host_weights(w, b):
    wbd = np.zeros((NSHIFT, 128, 32), np.float32)
    w = np.asarray(w, np.float32)
    for s, (ku, kv) in enumerate(SHIFTS):
        for kh in range(2):
            for kw in range(2):
                for ib in range(IB):
                    wbd[s, kh * 16 + kw * 8 + ib : 128 : 32, ib : 32 : 8] = (
                        w[:, :, ku, kv, kh, kw].T
                    )
    wext = np.zeros((128, WCOLS), np.uint16)
    wext[:, : NSHIFT * 32] = (
        wbd.transpose(1, 0, 2).reshape(128, NSHIFT * 32)
        .astype(ml_dtypes.bfloat16).view(np.uint16)
    )
    bias = np.tile(np.repeat(np.asarray(b, np.float32), IB), 4)
    wext[:, NSHIFT * 32 : NSHIFT * 32 + 2] = (
        bias.astype("<f4").view("<u2").reshape(128, 2)
    )
    return np.ascontiguousarray(wext).view(ml_dtypes.bfloat16)


def _host_shard(x):
    xp = np.pad(np.asarray(x, np.float32), ((0, 0), (0, 0), (1, 1), (0, 0), (0, 0), (0, 0)))
    shards = []
    for core in range(NCORES):
        bb, half = divmod(core, 2)
        xs = xp[bb, :, half * U : half * U + R]
        xs = xs.reshape(CIN, R, V, IO, IB, 2, J, 2)
        xs = xs.transpose(1, 0, 5, 7, 4, 2, 3, 6).astype(ml_dtypes.bfloat16)
        shards.append(np.ascontiguousarray(xs).reshape(R, 128, V, IO, J))
    return shards


def _build_program():
    nc = bacc.Bacc("TRN2", target_bir_lowering=False, debug=False)
    xs = nc.dram_tensor("xs", [R, 128, V, IO, J], BF16, kind="ExternalInput").ap()
    wbd = nc.dram_tensor("wbd", [128, WCOLS], BF16, kind="ExternalInput").ap()
    ys = nc.dram_tensor("ys", [U, 2, 128, VBS, IO, J], F16, kind="ExternalOutput").ap()

    with ExitStack() as ctx:
        zt = ctx.enter_context(nc.sbuf_tensor("zt", [128, NZ, V, IO, J], BF16))
        wt = ctx.enter_context(nc.sbuf_tensor("wt", [128, WCOLS], BF16))
        ot = ctx.enter_context(nc.sbuf_tensor("ot", [128, NOUT, VBS, IO, J], F16))
        junk = ctx.enter_context(nc.sbuf_tensor("junk", [128, 544], BF16))
        ps = [ctx.enter_context(nc.psum_tensor(f"ps{i}", [128, VBS, IO, J], F32)) for i in range(NPS)]
        # One semaphore per DMA; waited only at final values (16 per dma).
        sem_za = [ctx.enter_context(nc.semaphore(f"sem_za{r}")) for r in range(R)]
        sem_zb = [ctx.enter_context(nc.semaphore(f"sem_zb{r}")) for r in range(R)]
        sem_w = ctx.enter_context(nc.semaphore("sem_w"))
        sem_mm = ctx.enter_context(nc.semaphore("sem_mm"))
        sem_act = ctx.enter_context(nc.semaphore("sem_act"))
        sem_os = [ctx.enter_context(nc.semaphore(f"sem_o{g}")) for g in range(NG)]
        sem_warm = ctx.enter_context(nc.semaphore("sem_warm"))
        sem_junk = ctx.enter_context(nc.semaphore("sem_junk"))

        def emit_queue(eng, order):
            for item in order:
                if item == "w":
                    eng.dma_start(wt[:], wbd[:]).then_inc(sem_w, 16)
                elif item.startswith("o"):
                    g = int(item[1:])
                    u, rnd = divmod(g, 2)
                    eng.wait_ge(sem_act, g + 1)
                    eng.dma_start(ys[u, rnd], ot[:, g % NOUT]).then_inc(sem_os[g], 16)
                else:
                    r = int(item[1:-1])
                    ab = item[-1]
                    if r >= NZ:
                        eng.wait_ge(sem_mm, 2 * (r - NZ) + 2)
                    vs = slice(0, 17) if ab == "a" else slice(17, V)
                    sem = sem_za[r] if ab == "a" else sem_zb[r]
                    eng.dma_start(zt[:, r % NZ, vs], xs[r, :, vs]).then_inc(sem, 16)

        blk_ctx = nc.Block()
        block = blk_ctx.__enter__()

        @block.sync
        def _(sync):
            emit_queue(sync, Q_SYNC)

        @block.scalar
        def _(scalar):
            emit_queue(scalar, Q_SCAL)

        @block.tensor
        def _(tensor):
            # HAM pre-warm: keep the PE busy on scratch data while the first
            # z rows stream in, so real groups run at the warm 2.4 GHz clock.
            tensor.wait_ge(sem_junk, 1)
            last = None
            for k in range(NWARM):
                last = nc.tensor.matmul(
                    ps[NPS - 1][0:32],
                    junk[:, 0:32],
                    junk[:, 32 : 32 + 512],
                    start=(k == 0),
                    stop=(k == NWARM - 1),
                    skip_group_check=True,
                    tile_position=(0, 0),
                )
            last.then_inc(sem_warm)
            for g in range(NG):
                u, rnd = divmod(g, 2)
                if g == 0:
                    tensor.wait_ge(sem_w, 16)
                if g == NPS - 1:
                    tensor.wait_ge(sem_warm, 1)
                if g >= NPS:
                    tensor.wait_ge(sem_act, g - NPS + 1)
                psg = ps[g % NPS]
                sems = sem_za if rnd == 0 else sem_zb
                last = None
                for s, (ku, kv) in enumerate(SHIFTS):
                    # row u+ku wait right before its first use (ku-major);
                    # rows u, u+1 were waited by earlier groups except g0/g1
                    if s % 3 == 0 and (ku == 2 or g <= 1):
                        tensor.wait_ge(sems[u + ku], 16)
                    for c in range(4):
                        v0 = (rnd * 4 + c) * VBS
                        vv0 = max(0, 1 - kv - v0)
                        vv1 = min(VBS, V + 1 - kv - v0)
                        a = v0 + vv0 + kv - 1
                        last = nc.tensor.matmul(
                            psg[c * 32 : (c + 1) * 32, vv0:vv1, :, :],
                            wt[:, s * 32 : (s + 1) * 32],
                            zt[:, (u + ku) % NZ, a : a + (vv1 - vv0), :, :],
                            start=(s == 0),
                            stop=(s == NSHIFT - 1),
                            skip_group_check=True,
                            tile_position=(0, c * 32),
                        )
                last.then_inc(sem_mm)

        @block.vector
        def _(vector):
            nc.vector.memset(junk[:], 0).then_inc(sem_junk, 1)
            bt_ap = wt[:, NSHIFT * 32 : NSHIFT * 32 + 2].bitcast(F32)
            for g in range(NG):
                if g == 0:
                    vector.wait_ge(sem_w, 16)
                if g >= NOUT:
                    vector.wait_ge(sem_os[g - NOUT], 16)
                vector.wait_ge(sem_mm, g + 1)
                nc.vector.tensor_scalar(
                    ot[:, g % NOUT],
                    ps[g % NPS][:],
                    bt_ap,
                    None,
                    mybir.AluOpType.add,
                ).then_inc(sem_act)

        blk_ctx.__exit__(None, None, None)

    nc.compile()
    return nc


def _unshard(results):
    y = np.empty((B, COUT, D1, D2, I, J), np.float32)
    for core in range(NCORES):
        bb, half = divmod(core, 2)
        arr = results[core]["ys"].astype(np.float32).reshape(U, 2, 4, COUT, IB, VBS, IO, J)
        arr = arr.transpose(3, 0, 1, 2, 5, 6, 4, 7)
        y[bb, :, half * U : (half + 1) * U] = arr.reshape(COUT, U, V, I, J)
    return y


TRACE = False
LAST_RESULT = [None]


def kernel(x, w, b, _cache={}):
    if "nc" not in _cache:
        _cache["nc"] = _build_program()
    nc = _cache["nc"]
    wext = _host_weights(w, b)
    in_maps = [{"xs": xs, "wbd": wext} for xs in _host_shard(x)]
    res = run_bass_kernel_spmd(nc, in_maps, list(range(NCORES)), trace=TRACE)
    LAST_RESULT[0] = res
    return _unshard(res.results)
